# revision 22
# baseline (speedup 1.0000x reference)
"""Trainium2 Bass kernel for nn_MoE_90297392431448.

MoE layer: B=2, T=2048, D=1024, H=4096, E=8 experts, top-K=2 routing.

Strategy (expert-parallel, routed):
  - Host: gating softmax + top-2 selection in fp64 (tiny: 4096x1024 @ 1024x8),
    renormalized gate weights; gather each expert's tokens.
  - Device (8 cores, SPMD, one expert per core): two-stage FFN
        hT = gelu(W1.T @ xT + b1)        [H, M] (partitions = H-chunks)
        y  = (hT.T @ W2) * w             [M, D] (partitions = M-tiles)
    in bf16 matmuls with fp32 PSUM accumulation.
  - Host: scatter-add per-expert outputs (+ w*b2 rank-1 term) into [B,T,D].

Only the (data-dependent) routing/dispatch and the final scatter-add run on
host; >99.9% of FLOPs (the FFN matmuls + gelu) run on device.
"""

import numpy as np
import ml_dtypes

B, T, D, H, E, K = 2, 2048, 1024, 4096, 8, 2
P = 128
KD = D // P    # 8  k-chunks of the D contraction
CH = H // P    # 32 chunks of H
N_TOK = B * T

_compiled_cache = {}


def _m_blocks(M):
    """m blocks (offset, size), each at most 512 (one fp32 PSUM bank) and,
    when possible, at least 256: narrow matmuls are LDWEIGHTS-bound on the
    PE (~100ns floor), so a 128-wide block wastes ~half its issue slots.
    Big blocks first: a wide stage-1 block consumes W1 slowly (~0.15MB/us),
    leaving spare HBM bandwidth to stream W2 behind it."""
    blocks = []
    rem = M
    while rem:
        if rem in (128, 256, 384, 512):
            take = rem
        elif rem == 640:
            take = 384
        elif rem == M and rem >= 896:
            take = 384  # smaller first block -> less ramp-critical DMA
        else:
            take = 512
        blocks.append((M - rem, take))
        rem -= take
    return blocks


def _build(M):
    """Build + compile the per-expert FFN kernel for capacity M (mult of 128)."""
    import concourse.bass as bass
    import concourse.mybir as mybir
    import concourse.tile as tile
    from concourse import bacc

    bf16 = mybir.dt.bfloat16
    f32 = mybir.dt.float32

    nc = bacc.Bacc("TRN2", target_bir_lowering=False, debug=False, num_devices=E)

    # packed (SBUF-layout) DRAM images -> large contiguous DMA rows
    xt_d = nc.dram_tensor("xt", [P, KD * M], bf16, kind="ExternalInput")
    w1_d = nc.dram_tensor("w1", [P, KD * H], bf16, kind="ExternalInput")
    w2_d = nc.dram_tensor("w2", [P, CH * D], bf16, kind="ExternalInput")
    b1_d = nc.dram_tensor("b1s", [P, CH], f32, kind="ExternalInput")
    ws_d = nc.dram_tensor("ws", [P, M // P], f32, kind="ExternalInput")
    y_d = nc.dram_tensor("y", [M, D], f32, kind="ExternalOutput")

    m_blocks = _m_blocks(M)

    with tile.TileContext(nc) as tc:
        with (
            tc.tile_pool(name="weights", bufs=1) as wpool,
            tc.tile_pool(name="xin", bufs=1) as xpool,
            tc.tile_pool(name="hbuf", bufs=1) as hpool,
            tc.tile_pool(name="obuf", bufs=4) as opool,
            tc.tile_pool(name="ps1", bufs=3, space="PSUM") as psum1,
            tc.tile_pool(name="ps2", bufs=3, space="PSUM") as psum2,
            tc.tile_pool(name="warm", bufs=1, space="PSUM") as pswarm,
        ):
            # Warm up the PE clock (HAM un-throttles after ~3.4us of activity)
            # with dummy matmuls while the input DMAs stream in. Sized to
            # finish just before the first real matmul's data lands.
            wz = xpool.tile([P, 512], bf16, tag="warmsrc")
            nc.gpsimd.memset(wz[:], 0.0)
            pw_t = pswarm.tile([P, 512], f32, tag="warmps")
            for _ in range(17):
                nc.tensor.matmul(pw_t[:], wz[:, :P], wz[:], start=True, stop=True)
            # DMA issue order tuned for ramp: the first stage-1 psum group
            # needs all k-chunks of (xt block 0) + the first W1 column block.
            # All DRAM images are pre-packed on host into SBUF layout, so
            # every transfer is one contiguous row per partition.
            # Issue engines: sync = xt + W1, gpsimd = W2, scalar = biases.
            xt_sb = xpool.tile([P, KD * M], bf16)
            w1_sb = wpool.tile([P, KD * H], bf16)
            w2_sb = wpool.tile([P, CH * D], bf16)
            CB = 4096  # 1 MiB blocks
            mo0, mb0 = m_blocks[0]
            nc.sync.dma_start(xt_sb[:, :KD * mb0], xt_d.ap()[:, :KD * mb0])
            nc.sync.dma_start(w1_sb[:, :CB], w1_d.ap()[:, :CB])
            b1_sb = wpool.tile([P, CH], f32)
            nc.scalar.dma_start(b1_sb[:], b1_d.ap())
            ws_sb = wpool.tile([P, M // P], f32)
            nc.scalar.dma_start(ws_sb[:], ws_d.ap())
            for cb in range(1, KD * H // CB):
                nc.sync.dma_start(w1_sb[:, cb * CB:(cb + 1) * CB],
                                  w1_d.ap()[:, cb * CB:(cb + 1) * CB])
            if M > mb0:
                nc.sync.dma_start(xt_sb[:, KD * mb0:], xt_d.ap()[:, KD * mb0:])
            # W2 isn't needed until stage 2; gate it on first stage-1 output
            # so the ramp-critical transfers (xt b0 + W1) get full bandwidth,
            # then let it stream in stage 1's spare bandwidth.
            deferred = []
            for cb in range(CH * D // CB):
                deferred.append(
                    nc.gpsimd.dma_start(w2_sb[:, cb * CB:(cb + 1) * CB],
                                        w2_d.ap()[:, cb * CB:(cb + 1) * CB]))

            for bi, (mo, mb) in enumerate(m_blocks):
                fo = KD * mo
                # stage 1: hT[n, m] = gelu(sum_k W1[k, n] * x[m, k] + b1[n])
                ht = hpool.tile([P, CH, 512], bf16, tag="ht")
                for c in range(CH):
                    w1base = (c // 4) * 4096 + (c % 4) * P
                    ps = psum1.tile([P, 512], f32, tag="ps1")
                    for k in range(KD):
                        nc.tensor.matmul(
                            ps[:, :mb],
                            w1_sb[:, w1base + k * 512: w1base + k * 512 + P],
                            xt_sb[:, fo + k * mb: fo + (k + 1) * mb],
                            start=(k == 0),
                            stop=(k == KD - 1),
                        )
                    act = nc.scalar.activation(
                        ht[:, c, :mb], ps[:, :mb],
                        mybir.ActivationFunctionType.Gelu,
                        bias=b1_sb[:, c:c + 1],
                    )
                    if bi == 0 and c == 0 and deferred:
                        for dma in deferred:
                            tile.add_dep_helper(
                                dma.ins, act.ins,
                                reason="bulk DMA deferred past ramp")
                        deferred = []
                # stage 2: y[m, d] = w[m] * sum_h hT[h, m] * W2[h, d]
                for mt in range(mb // P):
                    mg = mo // P + mt
                    ot = opool.tile([P, D], f32, tag="ot")
                    for db in range(D // 512):
                        ps2 = psum2.tile([P, 512], f32, tag="ps2")
                        for c in range(CH):
                            nc.tensor.matmul(
                                ps2[:],
                                ht[:, c, mt * P:(mt + 1) * P],
                                w2_sb[:, c * D + db * 512: c * D + (db + 1) * 512],
                                start=(c == 0),
                                stop=(c == CH - 1),
                            )
                        nc.vector.tensor_scalar_mul(
                            ot[:, db * 512:(db + 1) * 512], ps2[:],
                            ws_sb[:, mg:mg + 1])
                    nc.sync.dma_start(
                        y_d.ap()[mo + mt * P: mo + (mt + 1) * P, :], ot[:])
    nc.compile()
    return nc


def _route(x2d, Wg, bg):
    """fp64 gating: returns (top2 indices [N,2], renormalized weights [N,2])."""
    logits = x2d.astype(np.float64) @ Wg.astype(np.float64) + bg.astype(np.float64)
    m = logits.max(-1, keepdims=True)
    e = np.exp(logits - m)
    gates = e / e.sum(-1, keepdims=True)
    top2 = np.argsort(-gates, axis=-1, kind="stable")[:, :K]
    g2 = np.take_along_axis(gates, top2, axis=-1)
    w2 = g2 / np.maximum(g2.sum(-1, keepdims=True), 1e-12)
    return top2, w2


def kernel(x, Wg, bg, W1, b1, W2, b2, _run_opts=None):
    from concourse.bass_utils import run_bass_kernel_spmd

    x = np.asarray(x)
    x2d = x.reshape(N_TOK, D)
    top2, wgt2 = _route(x2d, np.asarray(Wg), np.asarray(bg))

    # per-expert token lists
    pos = [np.where((top2 == e).any(-1))[0] for e in range(E)]
    pw = [
        (wgt2 * (top2 == e))[pos[e]].sum(-1).astype(np.float32)
        for e in range(E)
    ]
    max_n = max(len(p) for p in pos)
    M = max(P, -(-max_n // P) * P)

    if M not in _compiled_cache:
        _compiled_cache[M] = _build(M)
    nc = _compiled_cache[M]

    bf = ml_dtypes.bfloat16
    W1 = np.asarray(W1)
    W2 = np.asarray(W2)
    b1 = np.asarray(b1)
    b2 = np.asarray(b2)

    m_blocks = _m_blocks(M)

    in_maps = []
    for e in range(E):
        n_e = len(pos[e])
        xt = np.zeros((D, M), bf)
        xt[:, :n_e] = x2d[pos[e]].T.astype(bf)
        # pack xt -> [P, sum_b KD*mb] with per-block [k, m'] free layout
        xt3 = xt.reshape(KD, P, M)
        xtp = np.empty((P, KD * M), bf)
        for (mo, mb) in m_blocks:
            blk = xt3[:, :, mo:mo + mb]          # [KD, P, mb]
            xtp[:, KD * mo:KD * (mo + mb)] = (
                blk.transpose(1, 0, 2).reshape(P, KD * mb))
        # pack W1 [D, H] -> [P, KD*H]: free idx = cb*4096 + k*512 + h''
        w1p = (W1[e].astype(bf)
               .reshape(KD, P, H // 512, 512)      # [k, p, cb, h'']
               .transpose(1, 2, 0, 3)              # [p, cb, k, h'']
               .reshape(P, KD * H))
        # pack W2 [H, D] -> [P, CH*D]: free idx = c*D + d
        w2p = (W2[e].astype(bf)
               .reshape(CH, P, D)
               .transpose(1, 0, 2)
               .reshape(P, CH * D))
        w_pad = np.zeros((M,), np.float32)
        w_pad[:n_e] = pw[e]
        in_maps.append({
            "xt": np.ascontiguousarray(xtp),
            "w1": np.ascontiguousarray(w1p),
            "w2": np.ascontiguousarray(w2p),
            "b1s": np.ascontiguousarray(b1[e].reshape(CH, P).T.astype(np.float32)),
            "ws": np.ascontiguousarray(w_pad.reshape(M // P, P).T),
        })

    try:
        res = run_bass_kernel_spmd(nc, in_maps, core_ids=list(range(E)),
                                   **(_run_opts or {}))
    except Exception:
        # transient device errors (e.g. NRT_EXEC_UNIT_UNRECOVERABLE) have
        # been observed on this fabric; one retry usually clears them
        res = run_bass_kernel_spmd(nc, in_maps, core_ids=list(range(E)),
                                   **(_run_opts or {}))

    out = np.zeros((N_TOK, D), np.float32)
    for e in range(E):
        n_e = len(pos[e])
        if n_e == 0:
            continue
        y = res.results[e]["y"][:n_e]
        out[pos[e]] += y + pw[e][:, None] * b2[e][None, :].astype(np.float32)
    if _run_opts is not None:
        kernel._last_result = res
    return out.reshape(B, T, D)


if __name__ == "__main__":
    rng = np.random.default_rng(0)
    ins = {
        "x": rng.standard_normal((B, T, D), dtype=np.float32),
        "Wg": rng.standard_normal((D, E), dtype=np.float32) * 0.03,
        "bg": rng.standard_normal((E,), dtype=np.float32) * 0.03,
        "W1": rng.standard_normal((E, D, H), dtype=np.float32) * 0.03,
        "b1": rng.standard_normal((E, H), dtype=np.float32) * 0.03,
        "W2": rng.standard_normal((E, H, D), dtype=np.float32) * 0.015,
        "b2": rng.standard_normal((E, D), dtype=np.float32) * 0.015,
    }
    out = kernel(**ins)
    print("kernel out:", out.shape, out.dtype, float(np.abs(out).mean()))


# revision 23
# speedup vs baseline: 1.0207x; 1.0207x over previous
"""Trainium2 Bass kernel for nn_MoE_90297392431448.

MoE layer: B=2, T=2048, D=1024, H=4096, E=8 experts, top-K=2 routing.

Strategy (expert-parallel, routed):
  - Host: gating softmax + top-2 selection in fp64 (tiny: 4096x1024 @ 1024x8),
    renormalized gate weights; gather each expert's tokens.
  - Device (8 cores, SPMD, one expert per core): two-stage FFN
        hT = gelu(W1.T @ xT + b1)        [H, M] (partitions = H-chunks)
        y  = (hT.T @ W2) * w             [M, D] (partitions = M-tiles)
    in bf16 matmuls with fp32 PSUM accumulation.
  - Host: scatter-add per-expert outputs (+ w*b2 rank-1 term) into [B,T,D].

Only the (data-dependent) routing/dispatch and the final scatter-add run on
host; >99.9% of FLOPs (the FFN matmuls + gelu) run on device.
"""

import numpy as np
import ml_dtypes

B, T, D, H, E, K = 2, 2048, 1024, 4096, 8, 2
P = 128
KD = D // P    # 8  k-chunks of the D contraction
CH = H // P    # 32 chunks of H
N_TOK = B * T

_compiled_cache = {}


def _m_blocks(M):
    """m blocks (offset, size), each at most 512 (one fp32 PSUM bank) and,
    when possible, at least 256: narrow matmuls are LDWEIGHTS-bound on the
    PE (~100ns floor), so a 128-wide block wastes ~half its issue slots.
    Big blocks first: a wide stage-1 block consumes W1 slowly (~0.15MB/us),
    leaving spare HBM bandwidth to stream W2 behind it."""
    blocks = []
    rem = M
    while rem:
        if rem in (128, 256, 384, 512):
            take = rem
        elif rem == 640:
            take = 384
        else:
            take = 512
        blocks.append((M - rem, take))
        rem -= take
    return blocks


def _build(M):
    """Build + compile the per-expert FFN kernel for capacity M (mult of 128)."""
    import concourse.bass as bass
    import concourse.mybir as mybir
    import concourse.tile as tile
    from concourse import bacc

    bf16 = mybir.dt.bfloat16
    f32 = mybir.dt.float32

    nc = bacc.Bacc("TRN2", target_bir_lowering=False, debug=False, num_devices=E)

    # packed (SBUF-layout) DRAM images -> large contiguous DMA rows
    xt_d = nc.dram_tensor("xt", [P, KD * M], bf16, kind="ExternalInput")
    w1_d = nc.dram_tensor("w1", [P, KD * H], bf16, kind="ExternalInput")
    w2_d = nc.dram_tensor("w2", [P, CH * D], bf16, kind="ExternalInput")
    b1_d = nc.dram_tensor("b1s", [P, CH], f32, kind="ExternalInput")
    ws_d = nc.dram_tensor("ws", [P, M // P], f32, kind="ExternalInput")
    y_d = nc.dram_tensor("y", [M, D], f32, kind="ExternalOutput")

    m_blocks = _m_blocks(M)

    with tile.TileContext(nc) as tc:
        with (
            tc.tile_pool(name="weights", bufs=1) as wpool,
            tc.tile_pool(name="xin", bufs=1) as xpool,
            tc.tile_pool(name="hbuf", bufs=1) as hpool,
            tc.tile_pool(name="obuf", bufs=4) as opool,
            tc.tile_pool(name="ps1", bufs=3, space="PSUM") as psum1,
            tc.tile_pool(name="ps2", bufs=3, space="PSUM") as psum2,
            tc.tile_pool(name="warm", bufs=1, space="PSUM") as pswarm,
        ):
            # Warm up the PE clock (HAM un-throttles after ~3.4us of activity)
            # with dummy matmuls while the input DMAs stream in. Sized to
            # finish just before the first real matmul's data lands.
            wz = xpool.tile([P, 512], bf16, tag="warmsrc")
            nc.gpsimd.memset(wz[:], 0.0)
            pw_t = pswarm.tile([P, 512], f32, tag="warmps")
            for _ in range(18):
                nc.tensor.matmul(pw_t[:], wz[:, :P], wz[:], start=True, stop=True)
            # DMA issue order tuned for ramp: the first stage-1 psum group
            # needs all k-chunks of (xt block 0) + the first W1 column block.
            # All DRAM images are pre-packed on host into SBUF layout, so
            # every transfer is one contiguous row per partition.
            # Issue engines: sync = xt + W1, gpsimd = W2, scalar = biases.
            xt_sb = xpool.tile([P, KD * M], bf16)
            w1_sb = wpool.tile([P, KD * H], bf16)
            w2_sb = wpool.tile([P, CH * D], bf16)
            CB = 4096  # 1 MiB blocks
            mo0, mb0 = m_blocks[0]
            nc.sync.dma_start(xt_sb[:, :KD * mb0], xt_d.ap()[:, :KD * mb0])
            nc.sync.dma_start(w1_sb[:, :CB], w1_d.ap()[:, :CB])
            b1_sb = wpool.tile([P, CH], f32)
            nc.scalar.dma_start(b1_sb[:], b1_d.ap())
            ws_sb = wpool.tile([P, M // P], f32)
            nc.scalar.dma_start(ws_sb[:], ws_d.ap())
            for cb in range(1, KD * H // CB):
                nc.sync.dma_start(w1_sb[:, cb * CB:(cb + 1) * CB],
                                  w1_d.ap()[:, cb * CB:(cb + 1) * CB])
            if M > mb0:
                nc.sync.dma_start(xt_sb[:, KD * mb0:], xt_d.ap()[:, KD * mb0:])
            # W2 isn't needed until stage 2; gate it on first stage-1 output
            # so the ramp-critical transfers (xt b0 + W1) get full bandwidth,
            # then let it stream in stage 1's spare bandwidth.
            deferred = []
            for cb in range(CH * D // CB):
                deferred.append(
                    nc.gpsimd.dma_start(w2_sb[:, cb * CB:(cb + 1) * CB],
                                        w2_d.ap()[:, cb * CB:(cb + 1) * CB]))

            for bi, (mo, mb) in enumerate(m_blocks):
                fo = KD * mo
                # stage 1: hT[n, m] = gelu(sum_k W1[k, n] * x[m, k] + b1[n])
                ht = hpool.tile([P, CH, 512], bf16, tag="ht")
                for c in range(CH):
                    w1base = (c // 4) * 4096 + (c % 4) * P
                    ps = psum1.tile([P, 512], f32, tag="ps1")
                    for k in range(KD):
                        nc.tensor.matmul(
                            ps[:, :mb],
                            w1_sb[:, w1base + k * 512: w1base + k * 512 + P],
                            xt_sb[:, fo + k * mb: fo + (k + 1) * mb],
                            start=(k == 0),
                            stop=(k == KD - 1),
                        )
                    act = nc.scalar.activation(
                        ht[:, c, :mb], ps[:, :mb],
                        mybir.ActivationFunctionType.Gelu,
                        bias=b1_sb[:, c:c + 1],
                    )
                    if bi == 0 and c == 0 and deferred:
                        for dma in deferred:
                            tile.add_dep_helper(
                                dma.ins, act.ins,
                                reason="bulk DMA deferred past ramp")
                        deferred = []
                # stage 2: y[m, d] = w[m] * sum_h hT[h, m] * W2[h, d]
                for mt in range(mb // P):
                    mg = mo // P + mt
                    ot = opool.tile([P, D], f32, tag="ot")
                    for db in range(D // 512):
                        ps2 = psum2.tile([P, 512], f32, tag="ps2")
                        for c in range(CH):
                            nc.tensor.matmul(
                                ps2[:],
                                ht[:, c, mt * P:(mt + 1) * P],
                                w2_sb[:, c * D + db * 512: c * D + (db + 1) * 512],
                                start=(c == 0),
                                stop=(c == CH - 1),
                            )
                        nc.vector.tensor_scalar_mul(
                            ot[:, db * 512:(db + 1) * 512], ps2[:],
                            ws_sb[:, mg:mg + 1])
                    nc.sync.dma_start(
                        y_d.ap()[mo + mt * P: mo + (mt + 1) * P, :], ot[:])
    nc.compile()
    return nc


def _route(x2d, Wg, bg):
    """fp64 gating: returns (top2 indices [N,2], renormalized weights [N,2])."""
    logits = x2d.astype(np.float64) @ Wg.astype(np.float64) + bg.astype(np.float64)
    m = logits.max(-1, keepdims=True)
    e = np.exp(logits - m)
    gates = e / e.sum(-1, keepdims=True)
    top2 = np.argsort(-gates, axis=-1, kind="stable")[:, :K]
    g2 = np.take_along_axis(gates, top2, axis=-1)
    w2 = g2 / np.maximum(g2.sum(-1, keepdims=True), 1e-12)
    return top2, w2


def kernel(x, Wg, bg, W1, b1, W2, b2, _run_opts=None):
    from concourse.bass_utils import run_bass_kernel_spmd

    x = np.asarray(x)
    x2d = x.reshape(N_TOK, D)
    top2, wgt2 = _route(x2d, np.asarray(Wg), np.asarray(bg))

    # per-expert token lists
    pos = [np.where((top2 == e).any(-1))[0] for e in range(E)]
    pw = [
        (wgt2 * (top2 == e))[pos[e]].sum(-1).astype(np.float32)
        for e in range(E)
    ]
    max_n = max(len(p) for p in pos)
    M = max(P, -(-max_n // P) * P)

    if M not in _compiled_cache:
        _compiled_cache[M] = _build(M)
    nc = _compiled_cache[M]

    bf = ml_dtypes.bfloat16
    W1 = np.asarray(W1)
    W2 = np.asarray(W2)
    b1 = np.asarray(b1)
    b2 = np.asarray(b2)

    m_blocks = _m_blocks(M)

    in_maps = []
    for e in range(E):
        n_e = len(pos[e])
        xt = np.zeros((D, M), bf)
        xt[:, :n_e] = x2d[pos[e]].T.astype(bf)
        # pack xt -> [P, sum_b KD*mb] with per-block [k, m'] free layout
        xt3 = xt.reshape(KD, P, M)
        xtp = np.empty((P, KD * M), bf)
        for (mo, mb) in m_blocks:
            blk = xt3[:, :, mo:mo + mb]          # [KD, P, mb]
            xtp[:, KD * mo:KD * (mo + mb)] = (
                blk.transpose(1, 0, 2).reshape(P, KD * mb))
        # pack W1 [D, H] -> [P, KD*H]: free idx = cb*4096 + k*512 + h''
        w1p = (W1[e].astype(bf)
               .reshape(KD, P, H // 512, 512)      # [k, p, cb, h'']
               .transpose(1, 2, 0, 3)              # [p, cb, k, h'']
               .reshape(P, KD * H))
        # pack W2 [H, D] -> [P, CH*D]: free idx = c*D + d
        w2p = (W2[e].astype(bf)
               .reshape(CH, P, D)
               .transpose(1, 0, 2)
               .reshape(P, CH * D))
        w_pad = np.zeros((M,), np.float32)
        w_pad[:n_e] = pw[e]
        in_maps.append({
            "xt": np.ascontiguousarray(xtp),
            "w1": np.ascontiguousarray(w1p),
            "w2": np.ascontiguousarray(w2p),
            "b1s": np.ascontiguousarray(b1[e].reshape(CH, P).T.astype(np.float32)),
            "ws": np.ascontiguousarray(w_pad.reshape(M // P, P).T),
        })

    try:
        res = run_bass_kernel_spmd(nc, in_maps, core_ids=list(range(E)),
                                   **(_run_opts or {}))
    except Exception:
        # transient device errors (e.g. NRT_EXEC_UNIT_UNRECOVERABLE) have
        # been observed on this fabric; one retry usually clears them
        res = run_bass_kernel_spmd(nc, in_maps, core_ids=list(range(E)),
                                   **(_run_opts or {}))

    out = np.zeros((N_TOK, D), np.float32)
    for e in range(E):
        n_e = len(pos[e])
        if n_e == 0:
            continue
        y = res.results[e]["y"][:n_e]
        out[pos[e]] += y + pw[e][:, None] * b2[e][None, :].astype(np.float32)
    if _run_opts is not None:
        kernel._last_result = res
    return out.reshape(B, T, D)


if __name__ == "__main__":
    rng = np.random.default_rng(0)
    ins = {
        "x": rng.standard_normal((B, T, D), dtype=np.float32),
        "Wg": rng.standard_normal((D, E), dtype=np.float32) * 0.03,
        "bg": rng.standard_normal((E,), dtype=np.float32) * 0.03,
        "W1": rng.standard_normal((E, D, H), dtype=np.float32) * 0.03,
        "b1": rng.standard_normal((E, H), dtype=np.float32) * 0.03,
        "W2": rng.standard_normal((E, H, D), dtype=np.float32) * 0.015,
        "b2": rng.standard_normal((E, D), dtype=np.float32) * 0.015,
    }
    out = kernel(**ins)
    print("kernel out:", out.shape, out.dtype, float(np.abs(out).mean()))


# revision 25
# speedup vs baseline: 1.0259x; 1.0052x over previous
"""Trainium2 Bass kernel for nn_MoE_90297392431448.

MoE layer: B=2, T=2048, D=1024, H=4096, E=8 experts, top-K=2 routing.

Strategy (expert-parallel, routed):
  - Host: gating softmax + top-2 selection in fp64 (tiny: 4096x1024 @ 1024x8),
    renormalized gate weights; gather each expert's tokens.
  - Device (8 cores, SPMD, one expert per core): two-stage FFN
        hT = gelu(W1.T @ xT + b1)        [H, M] (partitions = H-chunks)
        y  = (hT.T @ W2) * w             [M, D] (partitions = M-tiles)
    in bf16 matmuls with fp32 PSUM accumulation.
  - Host: scatter-add per-expert outputs (+ w*b2 rank-1 term) into [B,T,D].

Only the (data-dependent) routing/dispatch and the final scatter-add run on
host; >99.9% of FLOPs (the FFN matmuls + gelu) run on device.
"""

import numpy as np
import ml_dtypes

B, T, D, H, E, K = 2, 2048, 1024, 4096, 8, 2
P = 128
KD = D // P    # 8  k-chunks of the D contraction
CH = H // P    # 32 chunks of H
N_TOK = B * T

_compiled_cache = {}


def _m_blocks(M):
    """m blocks (offset, size), each at most 512 (one fp32 PSUM bank) and,
    when possible, at least 256: narrow matmuls are LDWEIGHTS-bound on the
    PE (~100ns floor), so a 128-wide block wastes ~half its issue slots.
    Big blocks first: a wide stage-1 block consumes W1 slowly (~0.15MB/us),
    leaving spare HBM bandwidth to stream W2 behind it."""
    blocks = []
    rem = M
    while rem:
        if rem in (128, 256, 384, 512):
            take = rem
        elif rem == 640:
            take = 384
        else:
            take = 512
        blocks.append((M - rem, take))
        rem -= take
    return blocks


def _build(M):
    """Build + compile the per-expert FFN kernel for capacity M (mult of 128)."""
    import concourse.bass as bass
    import concourse.mybir as mybir
    import concourse.tile as tile
    from concourse import bacc

    bf16 = mybir.dt.bfloat16
    f32 = mybir.dt.float32

    nc = bacc.Bacc("TRN2", target_bir_lowering=False, debug=False, num_devices=E)

    # packed (SBUF-layout) DRAM images -> large contiguous DMA rows
    xt_d = nc.dram_tensor("xt", [P, KD * M], bf16, kind="ExternalInput")
    w1_d = nc.dram_tensor("w1", [P, KD * H], bf16, kind="ExternalInput")
    w2_d = nc.dram_tensor("w2", [P, CH * D], bf16, kind="ExternalInput")
    b1_d = nc.dram_tensor("b1s", [P, CH], f32, kind="ExternalInput")
    ws_d = nc.dram_tensor("ws", [P, M // P], f32, kind="ExternalInput")
    y_d = nc.dram_tensor("y", [M, D], f32, kind="ExternalOutput")

    m_blocks = _m_blocks(M)

    with tile.TileContext(nc) as tc:
        with (
            tc.tile_pool(name="weights", bufs=1) as wpool,
            tc.tile_pool(name="xin", bufs=1) as xpool,
            tc.tile_pool(name="hbuf", bufs=1) as hpool,
            tc.tile_pool(name="obuf", bufs=4) as opool,
            tc.tile_pool(name="ps1", bufs=3, space="PSUM") as psum1,
            tc.tile_pool(name="ps2", bufs=3, space="PSUM") as psum2,
            tc.tile_pool(name="warm", bufs=1, space="PSUM") as pswarm,
        ):
            # Warm up the PE clock (HAM un-throttles after ~3.4us of activity)
            # with dummy matmuls while the input DMAs stream in. Sized to
            # finish just before the first real matmul's data lands.
            wz = xpool.tile([P, 512], bf16, tag="warmsrc")
            nc.gpsimd.memset(wz[:], 0.0)
            pw_t = pswarm.tile([P, 512], f32, tag="warmps")
            for _ in range(9):
                nc.tensor.matmul(pw_t[:], wz[:, :P], wz[:], start=True, stop=True)
            # DMA issue order tuned for ramp: the first stage-1 psum group
            # needs all k-chunks of (xt block 0) + the first W1 column block.
            # All DRAM images are pre-packed on host into SBUF layout, so
            # every transfer is one contiguous row per partition.
            # Issue engines: sync = xt + W1, gpsimd = W2, scalar = biases.
            xt_sb = xpool.tile([P, KD * M], bf16)
            w1_sb = wpool.tile([P, KD * H], bf16)
            w2_sb = wpool.tile([P, CH * D], bf16)
            CB = 4096  # 1 MiB blocks
            mo0, mb0 = m_blocks[0]
            # first c-group's inputs split in k-halves: its k=0..3 matmuls
            # can start after ~1MB has landed instead of 2MB
            h1 = KD // 2 * mb0
            nc.sync.dma_start(xt_sb[:, :h1], xt_d.ap()[:, :h1])
            nc.sync.dma_start(w1_sb[:, :CB // 2], w1_d.ap()[:, :CB // 2])
            nc.sync.dma_start(xt_sb[:, h1:KD * mb0], xt_d.ap()[:, h1:KD * mb0])
            nc.sync.dma_start(w1_sb[:, CB // 2:CB], w1_d.ap()[:, CB // 2:CB])
            b1_sb = wpool.tile([P, CH], f32)
            nc.scalar.dma_start(b1_sb[:], b1_d.ap())
            ws_sb = wpool.tile([P, M // P], f32)
            nc.scalar.dma_start(ws_sb[:], ws_d.ap())
            for cb in range(1, KD * H // CB):
                nc.sync.dma_start(w1_sb[:, cb * CB:(cb + 1) * CB],
                                  w1_d.ap()[:, cb * CB:(cb + 1) * CB])
            if M > mb0:
                nc.sync.dma_start(xt_sb[:, KD * mb0:], xt_d.ap()[:, KD * mb0:])
            # W2 isn't needed until stage 2; gate it on first stage-1 output
            # so the ramp-critical transfers (xt b0 + W1) get full bandwidth,
            # then let it stream in stage 1's spare bandwidth.
            deferred = []
            for cb in range(CH * D // CB):
                deferred.append(
                    nc.gpsimd.dma_start(w2_sb[:, cb * CB:(cb + 1) * CB],
                                        w2_d.ap()[:, cb * CB:(cb + 1) * CB]))

            for bi, (mo, mb) in enumerate(m_blocks):
                fo = KD * mo
                # stage 1: hT[n, m] = gelu(sum_k W1[k, n] * x[m, k] + b1[n])
                ht = hpool.tile([P, CH, 512], bf16, tag="ht")
                for c in range(CH):
                    w1base = (c // 4) * 4096 + (c % 4) * P
                    ps = psum1.tile([P, 512], f32, tag="ps1")
                    for k in range(KD):
                        nc.tensor.matmul(
                            ps[:, :mb],
                            w1_sb[:, w1base + k * 512: w1base + k * 512 + P],
                            xt_sb[:, fo + k * mb: fo + (k + 1) * mb],
                            start=(k == 0),
                            stop=(k == KD - 1),
                        )
                    act = nc.scalar.activation(
                        ht[:, c, :mb], ps[:, :mb],
                        mybir.ActivationFunctionType.Gelu,
                        bias=b1_sb[:, c:c + 1],
                    )
                    if bi == 0 and c == 0 and deferred:
                        for dma in deferred:
                            tile.add_dep_helper(
                                dma.ins, act.ins,
                                reason="bulk DMA deferred past ramp")
                        deferred = []
                # stage 2: y[m, d] = w[m] * sum_h hT[h, m] * W2[h, d]
                for mt in range(mb // P):
                    mg = mo // P + mt
                    ot = opool.tile([P, D], f32, tag="ot")
                    for db in range(D // 512):
                        ps2 = psum2.tile([P, 512], f32, tag="ps2")
                        for c in range(CH):
                            nc.tensor.matmul(
                                ps2[:],
                                ht[:, c, mt * P:(mt + 1) * P],
                                w2_sb[:, c * D + db * 512: c * D + (db + 1) * 512],
                                start=(c == 0),
                                stop=(c == CH - 1),
                            )
                        nc.vector.tensor_scalar_mul(
                            ot[:, db * 512:(db + 1) * 512], ps2[:],
                            ws_sb[:, mg:mg + 1])
                    nc.sync.dma_start(
                        y_d.ap()[mo + mt * P: mo + (mt + 1) * P, :], ot[:])
    nc.compile()
    return nc


def _route(x2d, Wg, bg):
    """fp64 gating: returns (top2 indices [N,2], renormalized weights [N,2])."""
    logits = x2d.astype(np.float64) @ Wg.astype(np.float64) + bg.astype(np.float64)
    m = logits.max(-1, keepdims=True)
    e = np.exp(logits - m)
    gates = e / e.sum(-1, keepdims=True)
    top2 = np.argsort(-gates, axis=-1, kind="stable")[:, :K]
    g2 = np.take_along_axis(gates, top2, axis=-1)
    w2 = g2 / np.maximum(g2.sum(-1, keepdims=True), 1e-12)
    return top2, w2


def kernel(x, Wg, bg, W1, b1, W2, b2, _run_opts=None):
    from concourse.bass_utils import run_bass_kernel_spmd

    x = np.asarray(x)
    x2d = x.reshape(N_TOK, D)
    top2, wgt2 = _route(x2d, np.asarray(Wg), np.asarray(bg))

    # per-expert token lists
    pos = [np.where((top2 == e).any(-1))[0] for e in range(E)]
    pw = [
        (wgt2 * (top2 == e))[pos[e]].sum(-1).astype(np.float32)
        for e in range(E)
    ]
    max_n = max(len(p) for p in pos)
    M = max(P, -(-max_n // P) * P)

    if M not in _compiled_cache:
        _compiled_cache[M] = _build(M)
    nc = _compiled_cache[M]

    bf = ml_dtypes.bfloat16
    W1 = np.asarray(W1)
    W2 = np.asarray(W2)
    b1 = np.asarray(b1)
    b2 = np.asarray(b2)

    m_blocks = _m_blocks(M)

    in_maps = []
    for e in range(E):
        n_e = len(pos[e])
        xt = np.zeros((D, M), bf)
        xt[:, :n_e] = x2d[pos[e]].T.astype(bf)
        # pack xt -> [P, sum_b KD*mb] with per-block [k, m'] free layout
        xt3 = xt.reshape(KD, P, M)
        xtp = np.empty((P, KD * M), bf)
        for (mo, mb) in m_blocks:
            blk = xt3[:, :, mo:mo + mb]          # [KD, P, mb]
            xtp[:, KD * mo:KD * (mo + mb)] = (
                blk.transpose(1, 0, 2).reshape(P, KD * mb))
        # pack W1 [D, H] -> [P, KD*H]: free idx = cb*4096 + k*512 + h''
        w1p = (W1[e].astype(bf)
               .reshape(KD, P, H // 512, 512)      # [k, p, cb, h'']
               .transpose(1, 2, 0, 3)              # [p, cb, k, h'']
               .reshape(P, KD * H))
        # pack W2 [H, D] -> [P, CH*D]: free idx = c*D + d
        w2p = (W2[e].astype(bf)
               .reshape(CH, P, D)
               .transpose(1, 0, 2)
               .reshape(P, CH * D))
        w_pad = np.zeros((M,), np.float32)
        w_pad[:n_e] = pw[e]
        in_maps.append({
            "xt": np.ascontiguousarray(xtp),
            "w1": np.ascontiguousarray(w1p),
            "w2": np.ascontiguousarray(w2p),
            "b1s": np.ascontiguousarray(b1[e].reshape(CH, P).T.astype(np.float32)),
            "ws": np.ascontiguousarray(w_pad.reshape(M // P, P).T),
        })

    try:
        res = run_bass_kernel_spmd(nc, in_maps, core_ids=list(range(E)),
                                   **(_run_opts or {}))
    except Exception:
        # transient device errors (e.g. NRT_EXEC_UNIT_UNRECOVERABLE) have
        # been observed on this fabric; one retry usually clears them
        res = run_bass_kernel_spmd(nc, in_maps, core_ids=list(range(E)),
                                   **(_run_opts or {}))

    out = np.zeros((N_TOK, D), np.float32)
    for e in range(E):
        n_e = len(pos[e])
        if n_e == 0:
            continue
        y = res.results[e]["y"][:n_e]
        out[pos[e]] += y + pw[e][:, None] * b2[e][None, :].astype(np.float32)
    if _run_opts is not None:
        kernel._last_result = res
    return out.reshape(B, T, D)


if __name__ == "__main__":
    rng = np.random.default_rng(0)
    ins = {
        "x": rng.standard_normal((B, T, D), dtype=np.float32),
        "Wg": rng.standard_normal((D, E), dtype=np.float32) * 0.03,
        "bg": rng.standard_normal((E,), dtype=np.float32) * 0.03,
        "W1": rng.standard_normal((E, D, H), dtype=np.float32) * 0.03,
        "b1": rng.standard_normal((E, H), dtype=np.float32) * 0.03,
        "W2": rng.standard_normal((E, H, D), dtype=np.float32) * 0.015,
        "b2": rng.standard_normal((E, D), dtype=np.float32) * 0.015,
    }
    out = kernel(**ins)
    print("kernel out:", out.shape, out.dtype, float(np.abs(out).mean()))
